# revision 1
# baseline (speedup 1.0000x reference)
"""Distributed GCNConv kernel v2 for Trainium2 (8 NeuronCores).

Source-sharded (expert) parallelism: core k owns a contiguous slice of
SOURCE nodes. Each core computes h = (deg*x)@W for its own sources only
(plus a few "borrowed" sources for load balancing), gathers messages from
its LOCAL h table (single int16 window, no halo exchange), accumulates
partial sums for ALL destination tiles via one-hot matmuls, and the
partials are combined with a pipelined ReduceScatter collective (which
runs on the collective cores, off the DMA critical path). The owner of
each destination supertile applies deg_dst (+bias) and stores the output.

Load balancing: per destination tile, edge counts are equalized across
cores by reassigning surplus edges; a reassigned edge's source is added
to the receiving core's table ("borrowed" rows), so per-tile slot counts
are identical on every core and the SPMD program needs no cross-core
padding.

Self-contained: only needs numpy / ml_dtypes / the concourse Bass stack.
"""

import numpy as np
import ml_dtypes

import concourse.bacc as bacc
import concourse.bass as bass
import concourse.mybir as mybir
import concourse.tile as tile
from concourse.bass_utils import run_bass_kernel_spmd

P = 128
N_CORES = 8
ST = 8        # dst tiles per supertile (one PSUM bank per tile)
GA = 16       # node tiles per phase-A block
GMAX = 1792   # max gather indices per dma_gather call (< 2048 SWDGE ring limit)
MIN_SLOTS = 16  # min slots per dst tile (keeps empty/pad tiles alive)
BF16 = mybir.dt.bfloat16
F32 = mybir.dt.float32
I16 = mybir.dt.int16
npbf16 = ml_dtypes.bfloat16
PAD_ROWLOC = -1.0


def _wrap_idx(a):
    return np.tile(a.reshape(-1, 16).T, (8, 1))


def _plan(row, col, n_local, n_owned):
    own = -(-n_local // (N_CORES * P)) * P                # sources per core
    ntile = -(-n_owned // P)                              # real dst tiles
    n_sup = -(-ntile // ST)
    NG = -(-n_sup // N_CORES)                             # RS groups
    sup_pad = NG * N_CORES
    tile_pad = sup_pad * ST

    row = np.asarray(row).astype(np.int64)
    col = np.asarray(col).astype(np.int64)
    keep = row < n_owned
    r, c = row[keep], col[keep]
    core0 = c // own
    t = r // P

    # ---- balance edge counts per dst tile across cores ----
    counts = np.bincount(core0 * tile_pad + t,
                         minlength=N_CORES * tile_pad).reshape(N_CORES, tile_pad)
    total_t = counts.sum(axis=0)
    target = np.maximum(-(-total_t // N_CORES), MIN_SLOTS)
    cell = core0 * tile_pad + t
    order = np.lexsort((cell,))
    r, c, core0, t, cell = r[order], c[order], core0[order], t[order], cell[order]
    starts = np.concatenate(([0], np.cumsum(np.bincount(
        cell, minlength=N_CORES * tile_pad))))[:-1]
    rank = np.arange(len(cell)) - starts[cell]
    surplus = rank >= target[t]
    core = core0.copy()
    si = np.nonzero(surplus)[0]
    si = si[np.argsort(t[si], kind="stable")]
    deficit = np.maximum(target[None, :] - counts, 0)     # [N_CORES, tile_pad]
    n_sur = np.bincount(t[si], minlength=tile_pad)
    fill = []
    for ti in range(tile_pad):
        need = int(n_sur[ti])
        if not need:
            continue
        lst = np.repeat(np.arange(N_CORES), deficit[:, ti])
        assert len(lst) >= need, (ti, need, len(lst))
        fill.append(lst[:need])
    if fill:
        core[si] = np.concatenate(fill)

    # ---- per-core source tables (own + borrowed) ----
    borrowed = []
    for k in range(N_CORES):
        m = (core == k) & (core0 != k)
        borrowed.append(np.unique(c[m]))
    B_pad = -(-max(max((len(b) for b in borrowed), default=0), 1) // P) * P
    tbl_rows = own + B_pad
    TA2 = tbl_rows // P
    assert tbl_rows <= 32768, tbl_rows

    loc = np.where(core == core0, c - core * own, -1)
    for k in range(N_CORES):
        m = (core == k) & (loc < 0)
        if m.any():
            loc[m] = own + np.searchsorted(borrowed[k], c[m])
    trow = (loc % P) * TA2 + loc // P                     # partition-major

    # ---- stream layout: group-major supertile order ----
    sup_order = [NG * j + g for g in range(NG) for j in range(N_CORES)]
    stream_tiles = [s * ST + tl for s in sup_order for tl in range(ST)]
    stream_pos = {ti: i for i, ti in enumerate(stream_tiles)}
    cell_off = {}
    off = 0
    for ti in stream_tiles:
        cell_off[ti] = off
        off += int(target[ti])
    TOT = -(-off // GMAX) * GMAX if off % P else off      # 128-align via GMAX pad
    TOT = -(-off // P) * P
    NCHK = TOT // P

    t0_of_chunk = np.zeros(NCHK, dtype=np.int64)
    jobs = {}
    first_job = {}
    last_job = {}
    calls = [(a, min(GMAX, TOT - a)) for a in range(0, TOT, GMAX)]
    jlist_all = []
    for ti in stream_tiles:
        o, cnt = cell_off[ti], int(target[ti])
        for k in range(o // P, (o + cnt - 1) // P + 1):
            jlist_all.append((k, ti))
    jlist_all.sort(key=lambda kt: (kt[0], stream_pos[kt[1]]))
    seen = set()
    for (k, ti) in jlist_all:
        if k not in seen:
            seen.add(k)
            t0_of_chunk[k] = ti
    for (k, ti) in jlist_all:
        v = stream_pos[ti] - stream_pos[t0_of_chunk[k]]
        assert 0 <= v < 64, (ti, v)
        jobs.setdefault(k * P // GMAX, []).append((k, ti, v))
        if ti not in first_job:
            first_job[ti] = (k, ti)
        last_job[ti] = (k, ti)

    # ---- per-core packed idx / rowloc ----
    per_core = []
    spos_of_t = np.full(tile_pad, -1, dtype=np.int64)
    for ti, pos in stream_pos.items():
        spos_of_t[ti] = pos
    coff_of_t = np.zeros(tile_pad, dtype=np.int64)
    for ti, o in cell_off.items():
        coff_of_t[ti] = o
    t0pos_of_chunk = np.array([stream_pos[ti] for ti in t0_of_chunk])
    for k in range(N_CORES):
        m = core == k
        tk, rk, trk = t[m], r[m], trow[m]
        g = spos_of_t[tk]
        o2 = np.argsort(g, kind="stable")
        tk, rk, trk, g = tk[o2], rk[o2], trk[o2], g[o2]
        cnt = np.bincount(g, minlength=len(stream_tiles))
        st2 = np.concatenate(([0], np.cumsum(cnt)))[:-1]
        rank2 = np.arange(len(g)) - st2[g]
        dest = coff_of_t[tk] + rank2
        gidx = np.zeros(TOT, dtype=np.int16)   # pads gather row 0 (harmless)
        rloc = np.full(TOT, PAD_ROWLOC, dtype=np.float32)
        gidx[dest] = trk.astype(np.int16)
        rloc[dest] = (rk - tk * P + P * (spos_of_t[tk] -
                                         t0pos_of_chunk[dest // P])
                      ).astype(np.float32)
        per_core.append((_wrap_idx(gidx),
                         np.ascontiguousarray(rloc.reshape(NCHK, P).T)))

    plan = dict(n_local=n_local, n_owned=n_owned, own=own, B_pad=B_pad,
                tbl_rows=tbl_rows, TA2=TA2, NG=NG, sup_pad=sup_pad,
                tile_pad=tile_pad, TOT=TOT, NCHK=NCHK, calls=calls,
                jobs=jobs, first_job=first_job, last_job=last_job,
                sup_order=sup_order, borrowed=borrowed)
    return plan, per_core


def _build(plan, bias_zero=True):
    TA2, NG = plan["TA2"], plan["NG"]
    TOT, NCHK = plan["TOT"], plan["NCHK"]
    calls, jobs = plan["calls"], plan["jobs"]
    first_job, last_job = plan["first_job"], plan["last_job"]
    sup_order, sup_pad = plan["sup_order"], plan["sup_pad"]
    tbl_rows = plan["tbl_rows"]

    nc = bacc.Bacc("TRN2", target_bir_lowering=False, debug=False,
                   enable_asserts=False, num_devices=N_CORES)

    xt = nc.dram_tensor("xt", [P, TA2 * P], BF16, kind="ExternalInput")
    wgt = nc.dram_tensor("wgt", [P, P], BF16, kind="ExternalInput")
    iot = nc.dram_tensor("iot", [P, P], BF16, kind="ExternalInput")
    biasb = nc.dram_tensor("biasb", [P, P], F32, kind="ExternalInput")
    idx = nc.dram_tensor("idx", [P, TOT // 16], I16, kind="ExternalInput")
    rld = nc.dram_tensor("rl", [P, NCHK], F32, kind="ExternalInput")
    degd = nc.dram_tensor("degd", [P, NG * ST], F32, kind="ExternalInput")
    out = nc.dram_tensor("out", [NG * ST * P, P], F32, kind="ExternalOutput")
    hst = nc.dram_tensor("h_stash", [tbl_rows, P], BF16, kind="Internal")


    # RS ranges: front-loaded sizes so the tail collective is small
    sizes = []
    rem = NG
    while rem:
        sz = 3 if rem >= 5 else (2 if rem >= 3 else 1)
        sizes.append(sz)
        rem -= sz
    ranges = []        # (g0, Mr, blk0)
    g0 = 0
    for sz in sizes:
        ranges.append((g0, sz, g0 * N_CORES))
        g0 += sz
    rng_of_g = {}
    for ri, (gg, sz, _) in enumerate(ranges):
        for g in range(gg, gg + sz):
            rng_of_g[g] = ri
    # per-range partial/reduced tensors: keeps collective dependencies
    # range-local (no cross-range WAR through a shared tensor)
    ptabs = [nc.dram_tensor(f"ptab{ri}", [sz * N_CORES * ST * P, P], BF16,
                            kind="Internal") for ri, (_, sz, _) in enumerate(ranges)]
    rsouts = [nc.dram_tensor(f"rsout{ri}", [sz * ST * P, P], BF16,
                             kind="Internal") for ri, (_, sz, _) in enumerate(ranges)]
    # block layout within a range: 8 contiguous out-chunks each hold that
    # core's sz supertiles: local blk = j*sz + (g - gg)
    blk_of_sup = {}
    for s in sup_order:
        j, g = s // NG, s % NG
        gg, sz, b0 = ranges[rng_of_g[g]]
        blk_of_sup[s] = (rng_of_g[g], j * sz + (g - gg))

    with tile.TileContext(nc) as tc:
        with (
            tc.tile_pool(name="const", bufs=1) as constp,
            tc.tile_pool(name="xtp", bufs=9) as xtp,
            tc.tile_pool(name="hsb", bufs=6) as hsb,
            tc.tile_pool(name="ps", bufs=8, space="PSUM") as psp,
            tc.tile_pool(name="msgs", bufs=7) as msgsp,
            tc.tile_pool(name="sone", bufs=20) as sonep,
            tc.tile_pool(name="evac", bufs=8) as evacp,
            tc.tile_pool(name="fin", bufs=2) as finp,
        ):
            w_sb = constp.tile([P, P], BF16)
            nc.sync.dma_start(w_sb[:], wgt[:, :])
            iot_sb = constp.tile([P, P], BF16)
            bias_sb = constp.tile([P, P], F32)
            idx_sb = constp.tile([P, TOT // 16], I16)
            rl_sb = constp.tile([P, NCHK], F32)
            degd_sb = constp.tile([P, NG * ST], F32)

            # ---- phase A: local h = (x*deg) @ W, deg pre-folded on host ----
            # hoist ALL x-block loads: the load DMA runs back-to-back while
            # compute chases, instead of a per-block load->mm->store chain
            xts = []
            for b0 in range(0, TA2, GA):
                nt = min(GA, TA2 - b0)
                xt_t = xtp.tile([P, GA * P], BF16, tag="xt")
                nc.sync.dma_start(xt_t[:, :nt * P],
                                  xt[:, b0 * P:(b0 + nt) * P])
                xts.append(xt_t)
            for bi, b0 in enumerate(range(0, TA2, GA)):
                nt = min(GA, TA2 - b0)
                xt_t = xts[bi]
                h_t = hsb.tile([P, GA * P], BF16, tag="h")
                for j0 in range(0, nt, 4):
                    nj = min(4, nt - j0)
                    ps = psp.tile([P, 512], F32, tag="ps")
                    for cc in range(nj):
                        nc.tensor.matmul(
                            ps[:, cc * P:(cc + 1) * P],
                            xt_t[:, (j0 + cc) * P:(j0 + cc + 1) * P],
                            w_sb[:],
                        )
                    # deg is folded into x on the host; evac = one wide copy
                    if (j0 // 4) % 2 == 0:
                        nc.vector.tensor_scalar(
                            h_t[:, j0 * P:(j0 + nj) * P],
                            ps[:, :nj * P],
                            0.0, None, mybir.AluOpType.add,
                        )
                    else:
                        nc.scalar.activation(
                            h_t[:, j0 * P:(j0 + nj) * P],
                            ps[:, :nj * P],
                            mybir.ActivationFunctionType.Copy,
                        )
                hv = hst[:, :].rearrange("(p j) f -> p j f", p=P)[:, b0:b0 + nt, :]
                nc.sync.dma_start(
                    hv, h_t[:, :nt * P].rearrange("p (j f) -> p j f", f=P))

            # non-critical const + idx/rl loads issued after the phase-A x
            # loads (SP queue order): only w gates early compute
            nc.sync.dma_start(iot_sb[:], iot[:, :])
            nc.sync.dma_start(degd_sb[:], degd[:, :])
            nc.sync.dma_start(bias_sb[:], biasb[:, :])
            NIW = TOT // 16
            for q0 in range(0, NIW, -(-NIW // 6)):
                q1 = min(q0 + -(-NIW // 6), NIW)
                nc.sync.dma_start(idx_sb[:, q0:q1], idx[:, q0:q1])
            for q0 in range(0, NCHK, -(-NCHK // 4)):
                q1 = min(q0 + -(-NCHK // 4), NCHK)
                nc.sync.dma_start(rl_sb[:, q0:q1], rld[:, q0:q1])

            # ---- phase B ----
            pbs = {}        # s -> [ST psum tiles]
            ev = {}         # s -> [evac tile, tiles_done]
            stored = set()  # supertiles whose ptab store has been emitted
            cur_call = 0
            fin_pend = []   # ranges whose final pass runs after the stream

            def emit_rs(ri):
                nc.gpsimd.collective_compute(
                    "ReduceScatter",
                    mybir.AluOpType.add,
                    replica_groups=[list(range(N_CORES))],
                    ins=[ptabs[ri][:, :]],
                    outs=[rsouts[ri][:, :]],
                )

            def emit_final(ri):
                gg, sz, b0 = ranges[ri]
                fb = finp.tile([P, sz * ST * P], BF16, tag="finb")
                rv = rsouts[ri][:, :].rearrange("(m p t) f -> p m t f",
                                                p=P, m=sz)
                nc.sync.dma_start(
                    fb[:, :].rearrange("p (m t f) -> p m t f", f=P, m=sz), rv)
                ft = finp.tile([P, sz * ST * P], F32, tag="fin")
                for mm in range(sz):
                    g = gg + mm
                    for tl in range(ST):
                        cc0 = mm * ST + tl
                        # DVE, not Act: the evacuation stream owns Act at the
                        # end of the gather stream; finals must not delay it
                        nc.vector.tensor_scalar(
                            ft[:, cc0 * P:(cc0 + 1) * P],
                            fb[:, cc0 * P:(cc0 + 1) * P],
                            degd_sb[:, g * ST + tl:g * ST + tl + 1], None,
                            mybir.AluOpType.mult)
                        if not bias_zero:
                            nc.vector.tensor_tensor(
                                ft[:, cc0 * P:(cc0 + 1) * P],
                                ft[:, cc0 * P:(cc0 + 1) * P],
                                bias_sb[:], mybir.AluOpType.add)
                ov = out[gg * ST * P:(gg + sz) * ST * P, :].rearrange(
                    "(m p t) f -> p m t f", p=P, m=sz)
                nc.sync.dma_start(
                    ov, ft[:, :].rearrange("p (m t f) -> p m t f", f=P, m=sz))

            def finish_tile(s, tl):
                if s not in ev:
                    ev[s] = [evacp.tile([P, ST * P], BF16, tag="ev",
                                        name=f"ev{s}"), 0]
                et = ev[s][0]
                if tl % 2 == 1:   # 4:4 Act:DVE split (best with evac bufs=7)
                    nc.vector.tensor_scalar(
                        et[:, tl * P:(tl + 1) * P], pbs[s][tl][:],
                        0.0, None, mybir.AluOpType.add)
                else:
                    nc.scalar.activation(
                        et[:, tl * P:(tl + 1) * P], pbs[s][tl][:],
                        mybir.ActivationFunctionType.Copy)
                ev[s][1] += 1
                if ev[s][1] == ST:
                    ri_s, blk = blk_of_sup[s]
                    pv = ptabs[ri_s][blk * ST * P:(blk + 1) * ST * P,
                                     :].rearrange("(p t) f -> p t f", p=P)
                    nc.sync.dma_start(
                        pv, et[:, :].rearrange("p (t f) -> p t f", f=P))
                    del ev[s]
                    stored.add(s)
                    # schedule RS (and later the final pass) once the whole
                    # range is stored; deferred a couple of gather calls so
                    # the collective's sem-waits are already satisfied and
                    # don't head-of-line-block the Pool / SP queues
                    for ri, (gg, sz, _) in enumerate(ranges):
                        grp = [NG * j + g for j in range(N_CORES)
                               for g in range(gg, gg + sz)]
                        if s in grp and all(sp in stored for sp in grp):
                            emit_rs(ri)
                            fin_pend.append(ri)

            for ci, (o, L) in enumerate(calls):
                cur_call = ci
                nchk = L // P
                mg = msgsp.tile([P, (GMAX // P) * P], BF16, tag="mg")
                mg3 = mg[:, :nchk * P].rearrange("p (k f) -> p k f", f=P)
                nc.gpsimd.dma_gather(
                    mg3[:, :, :], hst[:, :],
                    idx_sb[:, o // 16:(o + L) // 16],
                    L, L, P, single_packet=False,
                )
                kbase = o // P
                for (k, tg, v) in jobs.get(ci, []):
                    s, tl = tg // ST, tg % ST
                    if s not in pbs:
                        pbs[s] = [psp.tile([P, P], F32, tag="ps",
                                           name=f"pb{s}_{i}")
                                  for i in range(ST)]
                    S_t = sonep.tile([P, P], BF16, tag="S")
                    nc.vector.tensor_scalar(
                        S_t[:], iot_sb[:],
                        float(v * P), rl_sb[:, k:k + 1],
                        mybir.AluOpType.add,
                        mybir.AluOpType.is_equal,
                    )
                    nc.tensor.matmul(
                        pbs[s][tl][:],
                        S_t[:], mg3[:, k - kbase, :],
                        start=(first_job[tg] == (k, tg)),
                        stop=(last_job[tg] == (k, tg)),
                    )
                    if last_job[tg] == (k, tg):
                        finish_tile(s, tl)

            for ri in fin_pend:
                emit_final(ri)

    nc.compile()
    return nc


def _pack_core(x, deg, plan, k):
    own, B_pad, TA2 = plan["own"], plan["B_pad"], plan["TA2"]
    n_local = plan["n_local"]
    rows = TA2 * P
    xp = np.zeros((rows, P), dtype=np.float32)
    lo, hi = k * own, min((k + 1) * own, n_local)
    xp[:hi - lo] = x[lo:hi] * deg[lo:hi, None]
    b = plan["borrowed"][k]
    if len(b):
        xp[own:own + len(b)] = x[b] * deg[b, None]
    return np.ascontiguousarray(xp.T.astype(npbf16))


def _pack_degd(deg, plan, k):
    NG, n_owned = plan["NG"], plan["n_owned"]
    d = np.zeros((NG * ST * P,), dtype=np.float32)
    for g in range(NG):
        s = NG * k + g
        lo = s * ST * P
        n = min(max(n_owned - lo, 0), ST * P)
        if n:
            # layout [p, g*ST+tl] -> value deg[lo + tl*P + p]
            blk = np.zeros((ST * P,), dtype=np.float32)
            blk[:n] = deg[lo:lo + n]
            d[g * ST * P:(g + 1) * ST * P] = blk
    return np.ascontiguousarray(d.reshape(NG * ST, P).T)


_CACHE = {}


def kernel(x, weight, bias, deg_inv_sqrt, row, col, num_owned,
           _want_trace=False):
    n_local = int(x.shape[0])
    n_owned = int(num_owned)
    x = np.asarray(x, dtype=np.float32)
    weight = np.asarray(weight, dtype=np.float32)
    bias = np.asarray(bias, dtype=np.float32)
    deg = np.asarray(deg_inv_sqrt, dtype=np.float32)

    plan, per_core = _plan(row, col, n_local, n_owned)
    bias_zero = bool(np.all(bias == 0.0))
    sig = (n_local, n_owned, plan["TOT"], plan["tbl_rows"], bias_zero)
    if sig in _CACHE:
        nc = _CACHE[sig]
    else:
        nc = _build(plan, bias_zero=bias_zero)
        _CACHE[sig] = nc

    wb = weight.astype(npbf16)
    iota = np.ascontiguousarray(
        np.broadcast_to(np.arange(P, dtype=np.float32), (P, P))).astype(npbf16)
    biasb = np.ascontiguousarray(np.broadcast_to(bias.astype(np.float32), (P, P)))

    in_maps = []
    for k in range(N_CORES):
        idxk, rlk = per_core[k]
        in_maps.append(dict(
            xt=_pack_core(x, deg, plan, k), wgt=wb, iot=iota, biasb=biasb,
            idx=np.ascontiguousarray(idxk), rl=rlk,
            degd=_pack_degd(deg, plan, k),
        ))

    res = run_bass_kernel_spmd(nc, in_maps, core_ids=list(range(N_CORES)),
                               trace=_want_trace)

    NG = plan["NG"]
    full = np.zeros((n_owned, P), dtype=np.float32)
    for k in range(N_CORES):
        o = np.asarray(res.results[k]["out"]).reshape(NG, P, ST, P)
        for g in range(NG):
            s = NG * k + g
            lo = s * ST * P
            if lo >= n_owned:
                continue
            blk = o[g].transpose(1, 0, 2).reshape(ST * P, P)
            n = min(n_owned - lo, ST * P)
            full[lo:lo + n] = blk[:n]
    kernel.last_results = res
    return full

